# revision 26
# baseline (speedup 1.0000x reference)
"""ArcFace margin loss (ArcMarginLoss) on 8 Trainium2 NeuronCores.

Classification-parallel sharding: V=32000 classes split across 8 cores
(4000 each, padded to 4096).  The device kernel is a pure fp8 GEMM +
exp-rowsum pipeline; everything O(B*D) or O(V*D)-elementwise lives on
the host.

Host prep (numpy):
  - x-hat = x/|x|, w-hat = W/|W| rows (fp32), scaled by 16 and cast to
    fp8 e4m3.  PSUM then holds 256*cos, and the exp stage applies scale
    s/256 and bias -s, computing exp(s*cos - 30) directly: since
    cos <= 1 no global max pass or cross-core collective is needed.
  - Both operands are packed K-major for the PE's fp8 DoubleRow mode
    (2x MAC throughput, ~216ns per 512-wide matmul = ~fp8 peak).
    Weights are packed piece-major ([P, piece, j, i, 512]) so each
    512-class piece is one contiguous-per-partition DMA.
  - The label-column path (cos_y, phi, per-row exp corrections) is
    O(B*D), computed on the host in fp64.

Device per core (one SPMD NEFF):
  - Flat [128, 4096] PSUM tensor managed as a manual 4-slot ring of
    [128, 1024] chunks (range-level dependency tracking gives each
    chunk drain ~3 chunk-periods of latency slack).  Chunk = (m-tile,
    class-half); each is filled by 4 DoubleRow matmuls.
  - Drains are split per m-tile across ALL THREE non-PE compute engines
    (the aggregate drain rate is what paces the kernel; GpSimd has no
    PSUM port so it only gets SBUF-side work):
      * 'S' m-tiles (18): scalar ACT Exp per chunk with fused accum
        row-sum, in place (~1.3us/chunk).
      * 'C' m-tiles (14): DVE tensor_scalar converts each psum chunk
        straight to int16 bf16 BIT CODES (Schraudolph exp in the bf16
        domain) in SBUF scratch (~1.2us/chunk, frees the psum slot
        fast); GpSimd tensor_tensor adds the two bf16 code half-rows
        (~2.1us, its queue is otherwise idle); one DVE
        tensor_scalar+accum_out pass over the halved codes yields the
        row-sum.  The DVE accum pass is DEFERRED until after the next
        chain's psum-freeing passes (strict-FIFO queue), and flushed
        early before each phase's last m-tile.
  - Two phases of 2 class-quarters each; the first two m-tiles of
    phase 0 are walked half-major to hide the initial weight-DMA
    latency, with DMA issues split across the sync+scalar HWDGE queues
    in need-order.
  - HAM warm-up: real DoubleRow matmuls on a zeroed fp8 tile ramp the
    PE clock gate (transposes don't count as PE-busy for HAM).
  - Output: per-(phase, m) row sums Spart [128, 4, 16] fp32, DMA'd
    per phase so only ~8KB trails the last matmul.

Host epilogue: S = sum_c sum_q S_cq, scatter-add the label corrections,
loss = mean(30 + log(S) - s*phi_label).
"""

import math
import numpy as np
from contextlib import ExitStack

import concourse.bass as bass
import concourse.tile as tile
from concourse import bacc, mybir
from concourse import bass_utils
from concourse._compat import with_exitstack

P = 128
B = 2048          # batch rows
D = 512           # feature dim
V = 32000         # classes
NCORES = 8
VS = V // NCORES  # 4000 classes per core
VSP = 4096        # padded shard size
MT = B // P       # 16 batch row tiles
NJ = 2            # DoubleRow passes over D (each contracts 256)
NQ = 4            # class quarters per core
NPIECE = 8        # 512-class weight pieces per core
PW = (512, 512, 512, 512, 512, 512, 512, 416)  # real cols per piece

S_SCALE = 30.0
M_MARGIN = 0.5
SHIFT = 30.0      # exp(logit - SHIFT): logits <= 30 so always <= 0
WS = 16.0         # fp8 encode scale for x-hat and w-hat
EPS = 1e-12
ESC = S_SCALE / (WS * WS)   # psum -> logit scale (psum = 256*cos)

F32 = mybir.dt.float32
BF16 = mybir.dt.bfloat16
F8 = mybir.dt.float8e4
I16 = mybir.dt.int16
OP = mybir.AluOpType
AF = mybir.ActivationFunctionType
AX = mybir.AxisListType
DR = mybir.MatmulPerfMode.DoubleRow

# int16 Schraudolph: code = round(A16*psum + B16); code bits ARE the bf16
# representation of ~exp(ESC*psum - SHIFT).  The -7.3616 recenters the
# 2^frac-vs-exp sawtooth so the value-weighted mean ratio is 1 (calibrated
# numerically; residual is ~1.8% noise/elem that averages out per row).
_LOG2E = 1.4426950408889634
A16 = float(np.float32(ESC * _LOG2E * 128.0))
B16 = float(np.float32(128.0 * (127.0 - SHIFT * _LOG2E) - 7.3616))

# drain-mode map per (phase, m): 'S' = wide scalar ACT-exp+accum,
# 'C' = DVE wide int16-Schraudolph TS + DVE fused accum pass,
# 'D' = DVE wide TS + scalar ACT-identity accum over the codes.
# Measured: S = 2.33+0.34us scalar; C = 1.78us + 2.09+0.10us DVE;
# D = 1.78us DVE + 2.00+0.34us scalar.  Tuned so both engines stay
# under the PE fill rate of ~2.07us per m-tile.
MODE = [
    ['C', 'S', 'C', 'S', 'C', 'S', 'C', 'S',
     'C', 'S', 'C', 'S', 'C', 'S', 'C', 'S'],
    ['S', 'C', 'S', 'C', 'S', 'C', 'S', 'C',
     'S', 'C', 'S', 'C', 'S', 'C', 'S', 'S'],
]
PREFIX_M = 2      # m-tiles of phase 0 walked q-major (DMA latency hiding)
N_WARM = 11        # HAM warm-up matmuls (N=512 DoubleRow each, ~0.5us)
TS2_FUSED = True  # use tensor_scalar+accum_out for the chain row-sum


@with_exitstack
def _arc_kernel(ctx: ExitStack, tc: tile.TileContext,
                xt_d: bass.AP, wt_d: bass.AP, s_d: bass.AP):
    nc = tc.nc

    sb = ctx.enter_context(tc.tile_pool(name="sb", bufs=1))
    scr_pool = ctx.enter_context(tc.tile_pool(name="scr", bufs=2))
    scr2_pool = ctx.enter_context(tc.tile_pool(name="scr2", bufs=3))
    # Flat PSUM: one [128, 4096] fp32 tensor = all 8 banks, managed as a
    # manual 4-slot ring of 1024-col chunks.  An m-tile's two chunks land
    # on adjacent slots (offsets 0+1024 or 2048+3072), so a single wide
    # scalar ACT can drain both and still release them with ~3
    # chunk-periods of latency slack (range-level dependency tracking).
    pmall = nc.alloc_psum_tensor("pmall", [P, 4096], F32).ap()

    xT = sb.tile([P, MT, NJ, 2, P], F8)        # [p, m, j, i, c]
    wT = sb.tile([P, NPIECE, NJ, 2, 512], F8)  # [p, piece, j, i, v]
    Spart = sb.tile([P, NQ, MT], F32)          # per-(ph, m) row sums
    warm = sb.tile([P, 2, 512], F8)            # zeros for HAM warm-up MMs
    zt = sb.tile([P, 1], F32)
    nbias = sb.tile([P, 1], F32)               # -SHIFT bias for the exp

    nc.gpsimd.memset(warm, 0.0)
    nc.vector.memset(Spart, 0.0)
    nc.vector.memset(zt, 0.0)
    nc.vector.memset(nbias, -SHIFT)

    # ---- DMA prefix -------------------------------------------------
    # Two hardware-DGE queues in parallel: sync carries most weight
    # pieces, scalar carries the first x slab + piece 2, so the first
    # m-tiles' operands land with minimum serialization.
    nc.sync.dma_start(out=wT[:, 0, 0], in_=wt_d[:, 0, 0])    # p0 j0
    nc.sync.dma_start(out=xT[:, 0:1], in_=xt_d[:, 0:1])      # x m0
    nc.sync.dma_start(out=wT[:, 1], in_=wt_d[:, 1])          # p1
    nc.sync.dma_start(out=wT[:, 3], in_=wt_d[:, 3])          # p3
    nc.sync.dma_start(out=xT[:, 2:6], in_=xt_d[:, 2:6])      # x m2-5
    nc.sync.dma_start(out=xT[:, 6:11], in_=xt_d[:, 6:11])    # x m6-10
    nc.sync.dma_start(out=xT[:, 11:16], in_=xt_d[:, 11:16])  # x m11-15
    nc.sync.dma_start(out=wT[:, 4], in_=wt_d[:, 4])          # p4
    nc.sync.dma_start(out=wT[:, 6], in_=wt_d[:, 6])          # p6

    nc.scalar.dma_start(out=wT[:, 0, 1], in_=wt_d[:, 0, 1])  # p0 j1
    nc.scalar.dma_start(out=xT[:, 1:2], in_=xt_d[:, 1:2])    # x m1
    nc.scalar.dma_start(out=wT[:, 2], in_=wt_d[:, 2])        # p2

    # Force the Exp table load now (~2.9us) so it's resident before the
    # first scalar drain at ~13us.
    e0 = sb.tile([P, 1], F32)
    nc.scalar.activation(out=e0, in_=zt, func=AF.Exp, bias=nbias)

    nc.scalar.dma_start(out=wT[:, 5], in_=wt_d[:, 5])        # p5
    nc.scalar.dma_start(out=wT[:, 7], in_=wt_d[:, 7])        # p7

    # HAM warm-up: real (zero-data) DoubleRow matmuls engage the PE
    # activity monitor while the first weight pieces land.
    for _ in range(N_WARM):
        nc.tensor.matmul(pmall[:, 0:512], warm[:, :, 0:128], warm,
                         start=True, stop=True, perf_mode=DR)

    # ---- chunk pipeline ---------------------------------------------
    # Chunk = (m, half) = [128, <=1024] window of the flat psum ring.
    def fill_chunk(base, ph, m, h):
        for s in (2 * h, 2 * h + 1):
            pc = 4 * ph + s
            w = PW[pc]
            lo = base + 512 * (s - 2 * h)
            for j in range(NJ):
                nc.tensor.matmul(
                    pmall[:, lo:lo + w],
                    xT[:, m, j],
                    wT[:, pc, j, :, 0:w],
                    start=(j == 0), stop=(j == NJ - 1),
                    perf_mode=DR)

    # Chain finishers: the gpsimd TT halves the codes right away (its
    # queue is otherwise idle); the DVE accum pass over the halved codes
    # is DEFERRED until after the NEXT chain's psum-freeing TS passes so
    # it never delays them on the DVE's strict-FIFO queue.
    pending = []

    def finish_chain(keep=0, on_scalar=False):
        while len(pending) > keep:
            ph, m, scr2, wq = pending.pop(0)
            if on_scalar:
                nc.scalar.activation(
                    out=scr2[:, 0:wq], in_=scr2[:, 0:wq],
                    func=AF.Identity, bias=zt,
                    accum_out=Spart[:, 2 * ph, m:m + 1])
                on_scalar = False
            else:
                nc.vector.tensor_scalar(
                    scr2[:, 0:wq], scr2[:, 0:wq], 1.0, 0.0,
                    OP.mult, OP.add,
                    accum_out=Spart[:, 2 * ph, m:m + 1])

    def chain_tail(ph, m, scr, wtot):
        """gpsimd-quarter the bf16 codes of chain (ph, m), defer the sum
        by two chains so it never waits on the TT latency."""
        wh = wtot // 2
        wq = wh // 2
        scr2 = scr2_pool.tile([P, 1024], BF16, name="scr2")
        nc.gpsimd.tensor_tensor(
            out=scr2[:, 0:wh], in0=scr[:, 0:wh].bitcast(BF16),
            in1=scr[:, wh:wtot].bitcast(BF16), op=OP.add)
        nc.gpsimd.tensor_tensor(
            out=scr2[:, 0:wq], in0=scr2[:, 0:wq],
            in1=scr2[:, wq:wh], op=OP.add)
        finish_chain(keep=1)
        pending.append((ph, m, scr2, wq))

    def drain_chunk(base, ph, m, h, scr, w):
        """Drain one [128, w] psum chunk of (m, half h) -- narrow ops."""
        if MODE[ph][m] == 'S':
            nc.scalar.activation(
                out=pmall[:, base:base + w], in_=pmall[:, base:base + w],
                func=AF.Exp, bias=nbias, scale=ESC,
                accum_out=Spart[:, 2 * ph + h, m:m + 1])
        else:
            nc.vector.tensor_scalar(scr[:, 1024 * h:1024 * h + w],
                                    pmall[:, base:base + w],
                                    A16, B16, OP.mult, OP.add)

    s_view = s_d.rearrange("(p q m) -> p q m", p=P, q=NQ)

    def chunk_w(ph, h):
        return PW[4 * ph + 2 * h] + PW[4 * ph + 2 * h + 1]

    chunk_idx = 0
    for ph in range(2):
        wtot = chunk_w(ph, 0) + chunk_w(ph, 1)
        if ph == 0:
            # prefix: m0/m1 walked half-major (q-major) to hide the
            # initial weight-piece DMA latency; narrow drains.
            scrs = [scr_pool.tile([P, 2048], I16, name=f"scr{m}")
                    if MODE[0][m] != 'S' else None for m in range(PREFIX_M)]
            for h in range(2):
                for m in range(PREFIX_M):
                    base = (chunk_idx % 4) * 1024
                    chunk_idx += 1
                    fill_chunk(base, 0, m, h)
                    drain_chunk(base, 0, m, h, scrs[m], chunk_w(0, h))
            for m in range(PREFIX_M):
                if MODE[0][m] != 'S':
                    chain_tail(0, m, scrs[m], wtot)
            ms = range(PREFIX_M, MT)
        else:
            ms = range(MT)
        for m in ms:
            base = (chunk_idx % 4) * 1024
            chunk_idx += 2
            if (ph == 0 and m == 13) or (ph == 1 and m == 12):
                # relieve the ~96%-busy DVE: emit the oldest pending
                # chain sum as a scalar ACT-identity accum instead
                finish_chain(keep=1, on_scalar=True)
            if m == MT - 1:
                # flush all pending chain sums so they overlap the last
                # m-tile instead of trailing the final matmul
                finish_chain(keep=0)
                if ph == 1:
                    # bulk-ship everything but the last m-tile's sums so
                    # only ~512B of output trails the final drain
                    nc.sync.dma_start(out=s_view[:, 2:4, 0:MT - 1],
                                      in_=Spart[:, 2:4, 0:MT - 1])
            scr = (scr_pool.tile([P, 2048], I16, name="scr")
                   if MODE[ph][m] != 'S' else None)
            fill_chunk(base, ph, m, 0)
            drain_chunk(base, ph, m, 0, scr, chunk_w(ph, 0))
            fill_chunk(base + 1024, ph, m, 1)
            drain_chunk(base + 1024, ph, m, 1, scr, chunk_w(ph, 1))
            if scr is not None:
                chain_tail(ph, m, scr, wtot)
        finish_chain()
        # ship this phase's sums while the next phase runs
        if ph == 0:
            nc.sync.dma_start(out=s_view[:, 0:2], in_=Spart[:, 0:2])
        else:
            nc.sync.dma_start(out=s_view[:, 2:4, MT - 1:MT],
                              in_=Spart[:, 2:4, MT - 1:MT])


def build_bass():
    nc = bacc.Bacc("TRN2", target_bir_lowering=False, debug=False,
                   enable_asserts=False, num_devices=NCORES)
    xt_d = nc.dram_tensor("xt_in", [P, MT, NJ, 2, P], F8,
                          kind="ExternalInput").ap()
    wt_d = nc.dram_tensor("wt_in", [P, NPIECE, NJ, 2, 512], F8,
                          kind="ExternalInput").ap()
    s_d = nc.dram_tensor("s_out", [NQ * B], F32, kind="ExternalOutput").ap()
    with tile.TileContext(nc) as tc:
        _arc_kernel(tc, xt_d, wt_d, s_d)
    nc.compile()
    return nc


_NC = None


def _get_nc():
    global _NC
    if _NC is None:
        _NC = build_bass()
    return _NC


def make_in_maps(xn: np.ndarray, W: np.ndarray):
    import ml_dtypes
    F8NP = ml_dtypes.float8_e4m3

    xq = (xn * WS).astype(F8NP)                      # [B, D]
    # xt[p, m, j, i, c] = xq[m*128 + c, j*256 + i*128 + p]
    xt = np.ascontiguousarray(
        xq.reshape(MT, P, NJ, 2, P).transpose(4, 0, 2, 3, 1))

    wnorm = np.linalg.norm(W, axis=1, keepdims=True)
    Wn = W / np.maximum(wnorm, EPS)
    in_maps = []
    for c in range(NCORES):
        wq = np.zeros((VSP, D), dtype=F8NP)
        wq[:VS] = (Wn[c * VS:(c + 1) * VS] * WS).astype(F8NP)
        # wt[p, piece, j, i, v] = wq[piece*512 + v, j*256 + i*128 + p]
        wt = np.ascontiguousarray(
            wq.reshape(NPIECE, 512, NJ, 2, P).transpose(4, 0, 2, 3, 1))
        in_maps.append({"xt_in": xt, "wt_in": wt})
    return in_maps, Wn


def kernel(x, W, labels, **run_kwargs):
    x = np.ascontiguousarray(np.asarray(x), dtype=np.float32)
    W = np.ascontiguousarray(np.asarray(W), dtype=np.float32)
    lab = np.asarray(labels).astype(np.int64)
    assert x.shape == (B, D) and W.shape == (V, D) and lab.shape == (B,), \
        (x.shape, W.shape, lab.shape)

    xn = x / np.maximum(np.linalg.norm(x, axis=1, keepdims=True), EPS)

    nc = _get_nc()
    in_maps, Wn = make_in_maps(xn, W)
    res = bass_utils.run_bass_kernel_spmd(
        nc, in_maps, core_ids=list(range(NCORES)), **run_kwargs)

    S = np.zeros(B, dtype=np.float64)
    for r in res.results:
        sp = r["s_out"].reshape(P, NQ, MT).sum(axis=1)  # add the quarters
        S += sp.T.reshape(-1).astype(np.float64)

    # Host label-column correction (O(B*D), fp64 epilogue).
    cos_y = np.einsum("bd,bd->b", xn.astype(np.float64),
                      Wn[lab].astype(np.float64))
    sin_y = np.sqrt(np.clip(1.0 - cos_y * cos_y, 0.0, 1.0))
    phi_y = cos_y * math.cos(M_MARGIN) - sin_y * math.sin(M_MARGIN)
    S += np.exp(S_SCALE * phi_y - SHIFT) - np.exp(S_SCALE * cos_y - SHIFT)
    loss = np.mean(SHIFT + np.log(S) - S_SCALE * phi_y)

    kernel.last_results = res
    return np.asarray(loss, dtype=np.float32)


# revision 28
# speedup vs baseline: 1.0119x; 1.0119x over previous
"""ArcFace margin loss (ArcMarginLoss) on 8 Trainium2 NeuronCores.

Classification-parallel sharding: V=32000 classes split across 8 cores
(4000 each, padded to 4096).  The device kernel is a pure fp8 GEMM +
exp-rowsum pipeline; everything O(B*D) or O(V*D)-elementwise lives on
the host.

Host prep (numpy):
  - x-hat = x/|x|, w-hat = W/|W| rows (fp32), scaled by 16 and cast to
    fp8 e4m3.  PSUM then holds 256*cos, and the exp stage applies scale
    s/256 and bias -s, computing exp(s*cos - 30) directly: since
    cos <= 1 no global max pass or cross-core collective is needed.
  - Both operands are packed K-major for the PE's fp8 DoubleRow mode
    (2x MAC throughput, ~216ns per 512-wide matmul = ~fp8 peak).
    Weights are packed piece-major ([P, piece, j, i, 512]) so each
    512-class piece is one contiguous-per-partition DMA.
  - The label-column path (cos_y, phi, per-row exp corrections) is
    O(B*D), computed on the host in fp64.

Device per core (one SPMD NEFF):
  - Flat [128, 4096] PSUM tensor managed as a manual 4-slot ring of
    [128, 1024] chunks (range-level dependency tracking gives each
    chunk drain ~3 chunk-periods of latency slack).  Chunk = (m-tile,
    class-half); each is filled by 4 DoubleRow matmuls.
  - Drains are split per m-tile across ALL THREE non-PE compute engines
    (the aggregate drain rate is what paces the kernel; GpSimd has no
    PSUM port so it only gets SBUF-side work):
      * 'S' m-tiles (18): scalar ACT Exp per chunk with fused accum
        row-sum, in place (~1.3us/chunk).
      * 'C' m-tiles (14): DVE tensor_scalar converts each psum chunk
        straight to int16 bf16 BIT CODES (Schraudolph exp in the bf16
        domain) in SBUF scratch (~1.2us/chunk, frees the psum slot
        fast); GpSimd tensor_tensor adds the two bf16 code half-rows
        (~2.1us, its queue is otherwise idle); one DVE
        tensor_scalar+accum_out pass over the halved codes yields the
        row-sum.  The DVE accum pass is DEFERRED until after the next
        chain's psum-freeing passes (strict-FIFO queue), and flushed
        early before each phase's last m-tile.
  - Two phases of 2 class-quarters each; the first two m-tiles of
    phase 0 are walked half-major to hide the initial weight-DMA
    latency, with DMA issues split across the sync+scalar HWDGE queues
    in need-order.
  - HAM warm-up: real DoubleRow matmuls on a zeroed fp8 tile ramp the
    PE clock gate (transposes don't count as PE-busy for HAM).
  - Output: per-(phase, m) row sums Spart [128, 4, 16] fp32, DMA'd
    per phase so only ~8KB trails the last matmul.

Host epilogue: S = sum_c sum_q S_cq, scatter-add the label corrections,
loss = mean(30 + log(S) - s*phi_label).
"""

import math
import numpy as np
from contextlib import ExitStack

import concourse.bass as bass
import concourse.tile as tile
from concourse import bacc, mybir
from concourse import bass_utils
from concourse._compat import with_exitstack

P = 128
B = 2048          # batch rows
D = 512           # feature dim
V = 32000         # classes
NCORES = 8
VS = V // NCORES  # 4000 classes per core
VSP = 4096        # padded shard size
MT = B // P       # 16 batch row tiles
NJ = 2            # DoubleRow passes over D (each contracts 256)
NQ = 4            # class quarters per core
NPIECE = 8        # 512-class weight pieces per core
PW = (512, 512, 512, 512, 512, 512, 512, 416)  # real cols per piece

S_SCALE = 30.0
M_MARGIN = 0.5
SHIFT = 30.0      # exp(logit - SHIFT): logits <= 30 so always <= 0
WS = 16.0         # fp8 encode scale for x-hat and w-hat
EPS = 1e-12
ESC = S_SCALE / (WS * WS)   # psum -> logit scale (psum = 256*cos)

F32 = mybir.dt.float32
BF16 = mybir.dt.bfloat16
F8 = mybir.dt.float8e4
I16 = mybir.dt.int16
OP = mybir.AluOpType
AF = mybir.ActivationFunctionType
AX = mybir.AxisListType
DR = mybir.MatmulPerfMode.DoubleRow

# int16 Schraudolph: code = round(A16*psum + B16); code bits ARE the bf16
# representation of ~exp(ESC*psum - SHIFT).  The -7.3616 recenters the
# 2^frac-vs-exp sawtooth so the value-weighted mean ratio is 1 (calibrated
# numerically; residual is ~1.8% noise/elem that averages out per row).
_LOG2E = 1.4426950408889634
A16 = float(np.float32(ESC * _LOG2E * 128.0))
B16 = float(np.float32(128.0 * (127.0 - SHIFT * _LOG2E) - 7.3616))

# drain-mode map per (phase, m): 'S' = wide scalar ACT-exp+accum,
# 'C' = DVE wide int16-Schraudolph TS + DVE fused accum pass,
# 'D' = DVE wide TS + scalar ACT-identity accum over the codes.
# Measured: S = 2.33+0.34us scalar; C = 1.78us + 2.09+0.10us DVE;
# D = 1.78us DVE + 2.00+0.34us scalar.  Tuned so both engines stay
# under the PE fill rate of ~2.07us per m-tile.
MODE = [
    ['C', 'S', 'C', 'S', 'C', 'S', 'C', 'S',
     'C', 'S', 'C', 'S', 'C', 'S', 'C', 'S'],
    ['S', 'C', 'S', 'C', 'S', 'C', 'S', 'C',
     'S', 'C', 'S', 'C', 'S', 'C', 'S', 'S'],
]
PREFIX_M = 2      # m-tiles of phase 0 walked q-major (DMA latency hiding)
N_WARM = 11        # HAM warm-up matmuls (N=512 DoubleRow each, ~0.5us)
TS2_FUSED = True  # use tensor_scalar+accum_out for the chain row-sum


@with_exitstack
def _arc_kernel(ctx: ExitStack, tc: tile.TileContext,
                xt_d: bass.AP, wt_d: bass.AP, s_d: bass.AP):
    nc = tc.nc

    sb = ctx.enter_context(tc.tile_pool(name="sb", bufs=1))
    scr_pool = ctx.enter_context(tc.tile_pool(name="scr", bufs=2))
    scr2_pool = ctx.enter_context(tc.tile_pool(name="scr2", bufs=3))
    # Flat PSUM: one [128, 4096] fp32 tensor = all 8 banks, managed as a
    # manual 4-slot ring of 1024-col chunks.  An m-tile's two chunks land
    # on adjacent slots (offsets 0+1024 or 2048+3072), so a single wide
    # scalar ACT can drain both and still release them with ~3
    # chunk-periods of latency slack (range-level dependency tracking).
    pmall = nc.alloc_psum_tensor("pmall", [P, 4096], F32).ap()

    xT = sb.tile([P, MT, NJ, 2, P], F8)        # [p, m, j, i, c]
    wT = sb.tile([P, NPIECE, NJ, 2, 512], F8)  # [p, piece, j, i, v]
    Spart = sb.tile([P, NQ, MT], F32)          # per-(ph, m) row sums
    warm = sb.tile([P, 2, 512], F8)            # zeros for HAM warm-up MMs
    zt = sb.tile([P, 1], F32)
    nbias = sb.tile([P, 1], F32)               # -SHIFT bias for the exp

    nc.gpsimd.memset(warm, 0.0)
    nc.vector.memset(Spart, 0.0)
    nc.vector.memset(zt, 0.0)
    nc.vector.memset(nbias, -SHIFT)

    # ---- DMA prefix -------------------------------------------------
    # Two hardware-DGE queues in parallel: sync carries most weight
    # pieces, scalar carries the first x slab + piece 2, so the first
    # m-tiles' operands land with minimum serialization.
    nc.sync.dma_start(out=wT[:, 0, 0], in_=wt_d[:, 0, 0])    # p0 j0
    nc.sync.dma_start(out=xT[:, 0:1], in_=xt_d[:, 0:1])      # x m0
    nc.sync.dma_start(out=wT[:, 1], in_=wt_d[:, 1])          # p1
    nc.sync.dma_start(out=wT[:, 3], in_=wt_d[:, 3])          # p3
    nc.sync.dma_start(out=xT[:, 2:6], in_=xt_d[:, 2:6])      # x m2-5
    nc.sync.dma_start(out=xT[:, 6:11], in_=xt_d[:, 6:11])    # x m6-10
    nc.sync.dma_start(out=xT[:, 11:16], in_=xt_d[:, 11:16])  # x m11-15
    nc.sync.dma_start(out=wT[:, 4], in_=wt_d[:, 4])          # p4
    nc.sync.dma_start(out=wT[:, 6], in_=wt_d[:, 6])          # p6

    nc.scalar.dma_start(out=wT[:, 0, 1], in_=wt_d[:, 0, 1])  # p0 j1
    nc.scalar.dma_start(out=xT[:, 1:2], in_=xt_d[:, 1:2])    # x m1
    nc.scalar.dma_start(out=wT[:, 2], in_=wt_d[:, 2])        # p2

    # Force the Exp table load now (~2.9us) so it's resident before the
    # first scalar drain at ~13us.
    e0 = sb.tile([P, 1], F32)
    nc.scalar.activation(out=e0, in_=zt, func=AF.Exp, bias=nbias)

    nc.scalar.dma_start(out=wT[:, 5], in_=wt_d[:, 5])        # p5
    nc.scalar.dma_start(out=wT[:, 7], in_=wt_d[:, 7])        # p7

    # HAM warm-up: real (zero-data) DoubleRow matmuls engage the PE
    # activity monitor while the first weight pieces land.
    for _ in range(N_WARM):
        nc.tensor.matmul(pmall[:, 0:512], warm[:, :, 0:128], warm,
                         start=True, stop=True, perf_mode=DR)

    # ---- chunk pipeline ---------------------------------------------
    # Chunk = (m, half) = [128, <=1024] window of the flat psum ring.
    def fill_chunk(base, ph, m, h):
        for s in (2 * h, 2 * h + 1):
            pc = 4 * ph + s
            w = PW[pc]
            lo = base + 512 * (s - 2 * h)
            for j in range(NJ):
                nc.tensor.matmul(
                    pmall[:, lo:lo + w],
                    xT[:, m, j],
                    wT[:, pc, j, :, 0:w],
                    start=(j == 0), stop=(j == NJ - 1),
                    perf_mode=DR)

    # Chain finishers: the gpsimd TT halves the codes right away (its
    # queue is otherwise idle); the DVE accum pass over the halved codes
    # is DEFERRED until after the NEXT chain's psum-freeing TS passes so
    # it never delays them on the DVE's strict-FIFO queue.
    pending = []

    def finish_chain(keep=0):
        while len(pending) > keep:
            ph, m, scr2, wq = pending.pop(0)
            nc.vector.tensor_scalar(
                scr2[:, 0:wq], scr2[:, 0:wq], 1.0, 0.0, OP.mult, OP.add,
                accum_out=Spart[:, 2 * ph, m:m + 1])

    def chain_tail(ph, m, scr, wtot):
        """gpsimd-quarter the bf16 codes of chain (ph, m), defer the sum
        by two chains so it never waits on the TT latency."""
        wh = wtot // 2
        wq = wh // 2
        scr2 = scr2_pool.tile([P, 1024], BF16, name="scr2")
        nc.gpsimd.tensor_tensor(
            out=scr2[:, 0:wh], in0=scr[:, 0:wh].bitcast(BF16),
            in1=scr[:, wh:wtot].bitcast(BF16), op=OP.add)
        nc.gpsimd.tensor_tensor(
            out=scr2[:, 0:wq], in0=scr2[:, 0:wq],
            in1=scr2[:, wq:wh], op=OP.add)
        finish_chain(keep=1)
        pending.append((ph, m, scr2, wq))

    def drain_chunk(base, ph, m, h, scr, w):
        """Drain one [128, w] psum chunk of (m, half h) -- narrow ops."""
        if MODE[ph][m] == 'S':
            nc.scalar.activation(
                out=pmall[:, base:base + w], in_=pmall[:, base:base + w],
                func=AF.Exp, bias=nbias, scale=ESC,
                accum_out=Spart[:, 2 * ph + h, m:m + 1])
        else:
            nc.vector.tensor_scalar(scr[:, 1024 * h:1024 * h + w],
                                    pmall[:, base:base + w],
                                    A16, B16, OP.mult, OP.add)

    s_view = s_d.rearrange("(p q m) -> p q m", p=P, q=NQ)

    def chunk_w(ph, h):
        return PW[4 * ph + 2 * h] + PW[4 * ph + 2 * h + 1]

    chunk_idx = 0
    for ph in range(2):
        wtot = chunk_w(ph, 0) + chunk_w(ph, 1)
        if ph == 0:
            # prefix: m0/m1 walked half-major (q-major) to hide the
            # initial weight-piece DMA latency; narrow drains.
            scrs = [scr_pool.tile([P, 2048], I16, name=f"scr{m}")
                    if MODE[0][m] != 'S' else None for m in range(PREFIX_M)]
            for h in range(2):
                for m in range(PREFIX_M):
                    base = (chunk_idx % 4) * 1024
                    chunk_idx += 1
                    fill_chunk(base, 0, m, h)
                    drain_chunk(base, 0, m, h, scrs[m], chunk_w(0, h))
            for m in range(PREFIX_M):
                if MODE[0][m] != 'S':
                    chain_tail(0, m, scrs[m], wtot)
            ms = range(PREFIX_M, MT)
        else:
            ms = range(MT)
        for m in ms:
            base = (chunk_idx % 4) * 1024
            chunk_idx += 2
            if m == MT - 1:
                # flush all pending chain sums so they overlap the last
                # m-tile instead of trailing the final matmul
                finish_chain(keep=0)
                if ph == 1:
                    # bulk-ship everything but the last m-tile's sums so
                    # only ~512B of output trails the final drain
                    nc.sync.dma_start(out=s_view[:, 2:4, 0:MT - 1],
                                      in_=Spart[:, 2:4, 0:MT - 1])
            scr = (scr_pool.tile([P, 2048], I16, name="scr")
                   if MODE[ph][m] != 'S' else None)
            fill_chunk(base, ph, m, 0)
            drain_chunk(base, ph, m, 0, scr, chunk_w(ph, 0))
            fill_chunk(base + 1024, ph, m, 1)
            drain_chunk(base + 1024, ph, m, 1, scr, chunk_w(ph, 1))
            if scr is not None:
                chain_tail(ph, m, scr, wtot)
        finish_chain()
        # ship this phase's sums while the next phase runs
        if ph == 0:
            nc.sync.dma_start(out=s_view[:, 0:2], in_=Spart[:, 0:2])
        else:
            nc.sync.dma_start(out=s_view[:, 2:4, MT - 1:MT],
                              in_=Spart[:, 2:4, MT - 1:MT])


def build_bass():
    nc = bacc.Bacc("TRN2", target_bir_lowering=False, debug=False,
                   enable_asserts=False, num_devices=NCORES)
    xt_d = nc.dram_tensor("xt_in", [P, MT, NJ, 2, P], F8,
                          kind="ExternalInput").ap()
    wt_d = nc.dram_tensor("wt_in", [P, NPIECE, NJ, 2, 512], F8,
                          kind="ExternalInput").ap()
    s_d = nc.dram_tensor("s_out", [NQ * B], F32, kind="ExternalOutput").ap()
    with tile.TileContext(nc) as tc:
        _arc_kernel(tc, xt_d, wt_d, s_d)
    nc.compile()
    return nc


_NC = None


def _get_nc():
    global _NC
    if _NC is None:
        _NC = build_bass()
    return _NC


def make_in_maps(xn: np.ndarray, W: np.ndarray):
    import ml_dtypes
    F8NP = ml_dtypes.float8_e4m3

    xq = (xn * WS).astype(F8NP)                      # [B, D]
    # xt[p, m, j, i, c] = xq[m*128 + c, j*256 + i*128 + p]
    xt = np.ascontiguousarray(
        xq.reshape(MT, P, NJ, 2, P).transpose(4, 0, 2, 3, 1))

    wnorm = np.linalg.norm(W, axis=1, keepdims=True)
    Wn = W / np.maximum(wnorm, EPS)
    in_maps = []
    for c in range(NCORES):
        wq = np.zeros((VSP, D), dtype=F8NP)
        wq[:VS] = (Wn[c * VS:(c + 1) * VS] * WS).astype(F8NP)
        # wt[p, piece, j, i, v] = wq[piece*512 + v, j*256 + i*128 + p]
        wt = np.ascontiguousarray(
            wq.reshape(NPIECE, 512, NJ, 2, P).transpose(4, 0, 2, 3, 1))
        in_maps.append({"xt_in": xt, "wt_in": wt})
    return in_maps, Wn


def kernel(x, W, labels, **run_kwargs):
    x = np.ascontiguousarray(np.asarray(x), dtype=np.float32)
    W = np.ascontiguousarray(np.asarray(W), dtype=np.float32)
    lab = np.asarray(labels).astype(np.int64)
    assert x.shape == (B, D) and W.shape == (V, D) and lab.shape == (B,), \
        (x.shape, W.shape, lab.shape)

    xn = x / np.maximum(np.linalg.norm(x, axis=1, keepdims=True), EPS)

    nc = _get_nc()
    in_maps, Wn = make_in_maps(xn, W)
    res = bass_utils.run_bass_kernel_spmd(
        nc, in_maps, core_ids=list(range(NCORES)), **run_kwargs)

    S = np.zeros(B, dtype=np.float64)
    for r in res.results:
        sp = r["s_out"].reshape(P, NQ, MT).sum(axis=1)  # add the quarters
        S += sp.T.reshape(-1).astype(np.float64)

    # Host label-column correction (O(B*D), fp64 epilogue).
    cos_y = np.einsum("bd,bd->b", xn.astype(np.float64),
                      Wn[lab].astype(np.float64))
    sin_y = np.sqrt(np.clip(1.0 - cos_y * cos_y, 0.0, 1.0))
    phi_y = cos_y * math.cos(M_MARGIN) - sin_y * math.sin(M_MARGIN)
    S += np.exp(S_SCALE * phi_y - SHIFT) - np.exp(S_SCALE * cos_y - SHIFT)
    loss = np.mean(SHIFT + np.log(S) - S_SCALE * phi_y)

    kernel.last_results = res
    return np.asarray(loss, dtype=np.float32)


# revision 30
# speedup vs baseline: 1.0198x; 1.0078x over previous
"""ArcFace margin loss (ArcMarginLoss) on 8 Trainium2 NeuronCores.

Classification-parallel sharding: V=32000 classes split across 8 cores
(4000 each, padded to 4096).  The device kernel is a pure fp8 GEMM +
exp-rowsum pipeline; everything O(B*D) or O(V*D)-elementwise lives on
the host.

Host prep (numpy):
  - x-hat = x/|x|, w-hat = W/|W| rows (fp32), scaled by 16 and cast to
    fp8 e4m3.  PSUM then holds 256*cos, and the exp stage applies scale
    s/256 and bias -s, computing exp(s*cos - 30) directly: since
    cos <= 1 no global max pass or cross-core collective is needed.
  - Both operands are packed K-major for the PE's fp8 DoubleRow mode
    (2x MAC throughput, ~216ns per 512-wide matmul = ~fp8 peak).
    Weights are packed piece-major ([P, piece, j, i, 512]) so each
    512-class piece is one contiguous-per-partition DMA.
  - The label-column path (cos_y, phi, per-row exp corrections) is
    O(B*D), computed on the host in fp64.

Device per core (one SPMD NEFF):
  - Flat [128, 4096] PSUM tensor managed as a manual 4-slot ring of
    [128, 1024] chunks (range-level dependency tracking gives each
    chunk drain ~3 chunk-periods of latency slack).  Chunk = (m-tile,
    class-half); each is filled by 4 DoubleRow matmuls.
  - Drains are split per m-tile across ALL THREE non-PE compute engines
    (the aggregate drain rate is what paces the kernel; GpSimd has no
    PSUM port so it only gets SBUF-side work):
      * 'S' m-tiles (18): scalar ACT Exp per chunk with fused accum
        row-sum, in place (~1.3us/chunk).
      * 'C' m-tiles (14): DVE tensor_scalar converts each psum chunk
        straight to int16 bf16 BIT CODES (Schraudolph exp in the bf16
        domain) in SBUF scratch (~1.2us/chunk, frees the psum slot
        fast); GpSimd tensor_tensor adds the two bf16 code half-rows
        (~2.1us, its queue is otherwise idle); one DVE
        tensor_scalar+accum_out pass over the halved codes yields the
        row-sum.  The DVE accum pass is DEFERRED until after the next
        chain's psum-freeing passes (strict-FIFO queue), and flushed
        early before each phase's last m-tile.
  - Two phases of 2 class-quarters each; the first two m-tiles of
    phase 0 are walked half-major to hide the initial weight-DMA
    latency, with DMA issues split across the sync+scalar HWDGE queues
    in need-order.
  - HAM warm-up: real DoubleRow matmuls on a zeroed fp8 tile ramp the
    PE clock gate (transposes don't count as PE-busy for HAM).
  - Output: per-(phase, m) row sums Spart [128, 4, 16] fp32, DMA'd
    per phase so only ~8KB trails the last matmul.

Host epilogue: S = sum_c sum_q S_cq, scatter-add the label corrections,
loss = mean(30 + log(S) - s*phi_label).
"""

import math
import numpy as np
from contextlib import ExitStack

import concourse.bass as bass
import concourse.tile as tile
from concourse import bacc, mybir
from concourse import bass_utils
from concourse._compat import with_exitstack

P = 128
B = 2048          # batch rows
D = 512           # feature dim
V = 32000         # classes
NCORES = 8
VS = V // NCORES  # 4000 classes per core
VSP = 4096        # padded shard size
MT = B // P       # 16 batch row tiles
NJ = 2            # DoubleRow passes over D (each contracts 256)
NQ = 4            # class quarters per core
NPIECE = 8        # 512-class weight pieces per core
PW = (512, 512, 512, 512, 512, 512, 512, 416)  # real cols per piece

S_SCALE = 30.0
M_MARGIN = 0.5
SHIFT = 30.0      # exp(logit - SHIFT): logits <= 30 so always <= 0
WS = 16.0         # fp8 encode scale for x-hat and w-hat
EPS = 1e-12
ESC = S_SCALE / (WS * WS)   # psum -> logit scale (psum = 256*cos)

F32 = mybir.dt.float32
BF16 = mybir.dt.bfloat16
F8 = mybir.dt.float8e4
I16 = mybir.dt.int16
OP = mybir.AluOpType
AF = mybir.ActivationFunctionType
AX = mybir.AxisListType
DR = mybir.MatmulPerfMode.DoubleRow

# int16 Schraudolph: code = round(A16*psum + B16); code bits ARE the bf16
# representation of ~exp(ESC*psum - SHIFT).  The -7.3616 recenters the
# 2^frac-vs-exp sawtooth so the value-weighted mean ratio is 1 (calibrated
# numerically; residual is ~1.8% noise/elem that averages out per row).
_LOG2E = 1.4426950408889634
A16 = float(np.float32(ESC * _LOG2E * 128.0))
B16 = float(np.float32(128.0 * (127.0 - SHIFT * _LOG2E) - 7.3616))

# drain-mode map per (phase, m): 'S' = wide scalar ACT-exp+accum,
# 'C' = DVE wide int16-Schraudolph TS + DVE fused accum pass,
# 'D' = DVE wide TS + scalar ACT-identity accum over the codes.
# Measured: S = 2.33+0.34us scalar; C = 1.78us + 2.09+0.10us DVE;
# D = 1.78us DVE + 2.00+0.34us scalar.  Tuned so both engines stay
# under the PE fill rate of ~2.07us per m-tile.
MODE = [
    ['C', 'S', 'C', 'S', 'C', 'S', 'C', 'S',
     'C', 'S', 'C', 'S', 'C', 'S', 'C', 'S'],
    ['S', 'C', 'S', 'C', 'S', 'C', 'S', 'C',
     'S', 'C', 'S', 'C', 'S', 'C', 'S', 'S'],
]
PREFIX_M = 2      # m-tiles of phase 0 walked q-major (DMA latency hiding)
N_WARM = 11        # HAM warm-up matmuls (N=512 DoubleRow each, ~0.5us)
TS2_FUSED = True  # use tensor_scalar+accum_out for the chain row-sum


@with_exitstack
def _arc_kernel(ctx: ExitStack, tc: tile.TileContext,
                xt_d: bass.AP, wt_d: bass.AP, s_d: bass.AP):
    nc = tc.nc

    sb = ctx.enter_context(tc.tile_pool(name="sb", bufs=1))
    scr_pool = ctx.enter_context(tc.tile_pool(name="scr", bufs=2))
    scr2_pool = ctx.enter_context(tc.tile_pool(name="scr2", bufs=3))
    # Flat PSUM: one [128, 4096] fp32 tensor = all 8 banks, managed as a
    # manual 4-slot ring of 1024-col chunks.  An m-tile's two chunks land
    # on adjacent slots (offsets 0+1024 or 2048+3072), so a single wide
    # scalar ACT can drain both and still release them with ~3
    # chunk-periods of latency slack (range-level dependency tracking).
    pmall = nc.alloc_psum_tensor("pmall", [P, 4096], F32).ap()

    xT = sb.tile([P, MT, NJ, 2, P], F8)        # [p, m, j, i, c]
    wT = sb.tile([P, NPIECE, NJ, 2, 512], F8)  # [p, piece, j, i, v]
    Spart = sb.tile([P, NQ, MT], F32)          # per-(ph, m) row sums
    warm = sb.tile([P, 2, 512], F8)            # zeros for HAM warm-up MMs
    zt = sb.tile([P, 1], F32)
    nbias = sb.tile([P, 1], F32)               # -SHIFT bias for the exp

    nc.gpsimd.memset(warm, 0.0)
    nc.vector.memset(Spart, 0.0)
    nc.vector.memset(zt, 0.0)
    nc.vector.memset(nbias, -SHIFT)

    # ---- DMA prefix -------------------------------------------------
    # Two hardware-DGE queues in parallel: sync carries most weight
    # pieces, scalar carries the first x slab + piece 2, so the first
    # m-tiles' operands land with minimum serialization.
    nc.sync.dma_start(out=wT[:, 0, 0], in_=wt_d[:, 0, 0])    # p0 j0
    nc.sync.dma_start(out=xT[:, 0:1], in_=xt_d[:, 0:1])      # x m0
    nc.sync.dma_start(out=wT[:, 1], in_=wt_d[:, 1])          # p1
    nc.sync.dma_start(out=wT[:, 3], in_=wt_d[:, 3])          # p3
    nc.sync.dma_start(out=xT[:, 2:6], in_=xt_d[:, 2:6])      # x m2-5
    nc.sync.dma_start(out=xT[:, 6:11], in_=xt_d[:, 6:11])    # x m6-10
    nc.sync.dma_start(out=xT[:, 11:16], in_=xt_d[:, 11:16])  # x m11-15
    nc.sync.dma_start(out=wT[:, 4], in_=wt_d[:, 4])          # p4
    nc.sync.dma_start(out=wT[:, 6], in_=wt_d[:, 6])          # p6

    nc.scalar.dma_start(out=wT[:, 0, 1], in_=wt_d[:, 0, 1])  # p0 j1
    nc.scalar.dma_start(out=xT[:, 1:2], in_=xt_d[:, 1:2])    # x m1
    nc.scalar.dma_start(out=wT[:, 2], in_=wt_d[:, 2])        # p2

    # Force the Exp table load now (~2.9us) so it's resident before the
    # first scalar drain at ~13us.
    e0 = sb.tile([P, 1], F32)
    nc.scalar.activation(out=e0, in_=zt, func=AF.Exp, bias=nbias)

    nc.scalar.dma_start(out=wT[:, 5], in_=wt_d[:, 5])        # p5
    nc.scalar.dma_start(out=wT[:, 7], in_=wt_d[:, 7])        # p7

    # HAM warm-up: real (zero-data) DoubleRow matmuls engage the PE
    # activity monitor while the first weight pieces land.
    for _ in range(N_WARM):
        nc.tensor.matmul(pmall[:, 0:512], warm[:, :, 0:128], warm,
                         start=True, stop=True, perf_mode=DR)

    # ---- chunk pipeline ---------------------------------------------
    # Chunk = (m, half) = [128, <=1024] window of the flat psum ring.
    def fill_chunk(base, ph, m, h):
        for s in (2 * h, 2 * h + 1):
            pc = 4 * ph + s
            w = PW[pc]
            lo = base + 512 * (s - 2 * h)
            for j in range(NJ):
                nc.tensor.matmul(
                    pmall[:, lo:lo + w],
                    xT[:, m, j],
                    wT[:, pc, j, :, 0:w],
                    start=(j == 0), stop=(j == NJ - 1),
                    perf_mode=DR)

    # Chain finishers: the gpsimd TT halves the codes right away (its
    # queue is otherwise idle); the DVE accum pass over the halved codes
    # is DEFERRED until after the NEXT chain's psum-freeing TS passes so
    # it never delays them on the DVE's strict-FIFO queue.
    pending = []

    def finish_chain(keep=0):
        while len(pending) > keep:
            ph, m, scr2, wq = pending.pop(0)
            nc.vector.tensor_scalar(
                scr2[:, 0:wq], scr2[:, 0:wq], 1.0, 0.0, OP.mult, OP.add,
                accum_out=Spart[:, 2 * ph, m:m + 1])

    def chain_tail(ph, m, scr, wtot):
        """gpsimd-quarter the bf16 codes of chain (ph, m), defer the sum
        by two chains so it never waits on the TT latency."""
        wh = wtot // 2
        wq = wh // 2
        scr2 = scr2_pool.tile([P, 1024], BF16, name="scr2")
        nc.gpsimd.tensor_tensor(
            out=scr2[:, 0:wh], in0=scr[:, 0:wh].bitcast(BF16),
            in1=scr[:, wh:wtot].bitcast(BF16), op=OP.add)
        nc.gpsimd.tensor_tensor(
            out=scr2[:, 0:wq], in0=scr2[:, 0:wq],
            in1=scr2[:, wq:wh], op=OP.add)
        finish_chain(keep=1)
        pending.append((ph, m, scr2, wq))

    def drain_chunk(base, ph, m, h, scr, w):
        """Drain one [128, w] psum chunk of (m, half h) -- narrow ops."""
        if MODE[ph][m] == 'S':
            nc.scalar.activation(
                out=pmall[:, base:base + w], in_=pmall[:, base:base + w],
                func=AF.Exp, bias=nbias, scale=ESC,
                accum_out=Spart[:, 2 * ph + h, m:m + 1])
        else:
            nc.vector.tensor_scalar(scr[:, 1024 * h:1024 * h + w],
                                    pmall[:, base:base + w],
                                    A16, B16, OP.mult, OP.add)

    s_view = s_d.rearrange("(p q m) -> p q m", p=P, q=NQ)

    def chunk_w(ph, h):
        return PW[4 * ph + 2 * h] + PW[4 * ph + 2 * h + 1]

    chunk_idx = 0
    for ph in range(2):
        wtot = chunk_w(ph, 0) + chunk_w(ph, 1)
        if ph == 0:
            # prefix: m0/m1 interleaved at 512-piece granularity so the
            # PE starts on piece-0 matmuls as soon as it lands and later
            # pieces stream in just ahead of their use; narrow drains.
            scrs = [scr_pool.tile([P, 2048], I16, name=f"scr{m}")
                    if MODE[0][m] != 'S' else None for m in range(PREFIX_M)]
            for sp in range(4):
                w = PW[sp]
                for m in range(PREFIX_M):
                    lo = 2048 * m + 512 * sp
                    for j in range(NJ):
                        nc.tensor.matmul(
                            pmall[:, lo:lo + w],
                            xT[:, m, j],
                            wT[:, sp, j, :, 0:w],
                            start=(j == 0), stop=(j == NJ - 1),
                            perf_mode=DR)
                if sp % 2 == 1:
                    h = sp // 2
                    for m in range(PREFIX_M):
                        drain_chunk(2048 * m + 1024 * h, 0, m, h,
                                    scrs[m], chunk_w(0, h))
            chunk_idx += 2 * PREFIX_M
            for m in range(PREFIX_M):
                if MODE[0][m] != 'S':
                    chain_tail(0, m, scrs[m], wtot)
            ms = range(PREFIX_M, MT)
        else:
            ms = range(MT)
        for m in ms:
            base = (chunk_idx % 4) * 1024
            chunk_idx += 2
            if m == MT - 1:
                # flush all pending chain sums so they overlap the last
                # m-tile instead of trailing the final matmul
                finish_chain(keep=0)
            scr = (scr_pool.tile([P, 2048], I16, name="scr")
                   if MODE[ph][m] != 'S' else None)
            fill_chunk(base, ph, m, 0)
            drain_chunk(base, ph, m, 0, scr, chunk_w(ph, 0))
            fill_chunk(base + 1024, ph, m, 1)
            drain_chunk(base + 1024, ph, m, 1, scr, chunk_w(ph, 1))
            if scr is not None:
                chain_tail(ph, m, scr, wtot)
        finish_chain()
        # ship this phase's sums while the next phase runs
        nc.sync.dma_start(out=s_view[:, 2 * ph:2 * ph + 2],
                          in_=Spart[:, 2 * ph:2 * ph + 2])


def build_bass():
    nc = bacc.Bacc("TRN2", target_bir_lowering=False, debug=False,
                   enable_asserts=False, num_devices=NCORES)
    xt_d = nc.dram_tensor("xt_in", [P, MT, NJ, 2, P], F8,
                          kind="ExternalInput").ap()
    wt_d = nc.dram_tensor("wt_in", [P, NPIECE, NJ, 2, 512], F8,
                          kind="ExternalInput").ap()
    s_d = nc.dram_tensor("s_out", [NQ * B], F32, kind="ExternalOutput").ap()
    with tile.TileContext(nc) as tc:
        _arc_kernel(tc, xt_d, wt_d, s_d)
    nc.compile()
    return nc


_NC = None


def _get_nc():
    global _NC
    if _NC is None:
        _NC = build_bass()
    return _NC


def make_in_maps(xn: np.ndarray, W: np.ndarray):
    import ml_dtypes
    F8NP = ml_dtypes.float8_e4m3

    xq = (xn * WS).astype(F8NP)                      # [B, D]
    # xt[p, m, j, i, c] = xq[m*128 + c, j*256 + i*128 + p]
    xt = np.ascontiguousarray(
        xq.reshape(MT, P, NJ, 2, P).transpose(4, 0, 2, 3, 1))

    wnorm = np.linalg.norm(W, axis=1, keepdims=True)
    Wn = W / np.maximum(wnorm, EPS)
    in_maps = []
    for c in range(NCORES):
        wq = np.zeros((VSP, D), dtype=F8NP)
        wq[:VS] = (Wn[c * VS:(c + 1) * VS] * WS).astype(F8NP)
        # wt[p, piece, j, i, v] = wq[piece*512 + v, j*256 + i*128 + p]
        wt = np.ascontiguousarray(
            wq.reshape(NPIECE, 512, NJ, 2, P).transpose(4, 0, 2, 3, 1))
        in_maps.append({"xt_in": xt, "wt_in": wt})
    return in_maps, Wn


def kernel(x, W, labels, **run_kwargs):
    x = np.ascontiguousarray(np.asarray(x), dtype=np.float32)
    W = np.ascontiguousarray(np.asarray(W), dtype=np.float32)
    lab = np.asarray(labels).astype(np.int64)
    assert x.shape == (B, D) and W.shape == (V, D) and lab.shape == (B,), \
        (x.shape, W.shape, lab.shape)

    xn = x / np.maximum(np.linalg.norm(x, axis=1, keepdims=True), EPS)

    nc = _get_nc()
    in_maps, Wn = make_in_maps(xn, W)
    res = bass_utils.run_bass_kernel_spmd(
        nc, in_maps, core_ids=list(range(NCORES)), **run_kwargs)

    S = np.zeros(B, dtype=np.float64)
    for r in res.results:
        sp = r["s_out"].reshape(P, NQ, MT).sum(axis=1)  # add the quarters
        S += sp.T.reshape(-1).astype(np.float64)

    # Host label-column correction (O(B*D), fp64 epilogue).
    cos_y = np.einsum("bd,bd->b", xn.astype(np.float64),
                      Wn[lab].astype(np.float64))
    sin_y = np.sqrt(np.clip(1.0 - cos_y * cos_y, 0.0, 1.0))
    phi_y = cos_y * math.cos(M_MARGIN) - sin_y * math.sin(M_MARGIN)
    S += np.exp(S_SCALE * phi_y - SHIFT) - np.exp(S_SCALE * cos_y - SHIFT)
    loss = np.mean(SHIFT + np.log(S) - S_SCALE * phi_y)

    kernel.last_results = res
    return np.asarray(loss, dtype=np.float32)
